# revision 1
# baseline (speedup 1.0000x reference)
"""GQA causal self-attention (sliding window 1024 + 4-token sink) on 8 trn2
NeuronCores.

Sharding: data parallel on batch (2) x tensor parallel on kv-head groups (4).
Core c handles batch c//4 and kv head c%4 (query heads 4g..4g+3): wq/wk/wv are
split column-wise (rows of the [out,in] weights), wo row-wise; each core
produces a [C,T] partial of the output projection and the host sums the 4
partials per batch.

Per-core kernel (all matmul operands bf16, fp32 PSUM accumulation):
  qT/kT/vT = W^T.T @ xT on the PE (outputs kept transposed [d,t] so attention
  scores can be computed without any transposes), RoPE applied in [d,t] layout
  via a half-swap permutation matmul + elementwise combine, scores S^T[tj,ti]
  per 128-wide key tile against the 9-tile sliding window + sink, masked by
  zeroing exp(S^T) blocks in SBUF (gpsimd affine_select), softmax without
  max-subtraction (|scale*S| <= ~6 for this distribution), denominators via a
  ones-vector matmul riding the same PT stream, y^T accumulated in PSUM and
  column-scaled by 1/sum, then the wo row-block matmul emits outT = partial^T.
"""

import os
import sys

import numpy as np
import ml_dtypes

sys.path.insert(0, "/opt/trn_rl_repo")

import orjson

import concourse.bass as bass
import concourse.tile as tile
from concourse import mybir
from concourse.bass_utils import run_bass_kernel_spmd

# ---------------------------------------------------------------------------
# Workarounds for the walrus build in this container: it rejects more than one
# sync-wait per instruction (setupSyncWait on the *_NO_STRUCT encodings).
# 1) TileContext's final drain carries one wait per live proc -> put each wait
#    on its own NoOp ahead of a clean drain.
# 2) Any scheduled instruction can end up with >1 waits -> post-process the
#    serialized BIR and hoist extra waits onto single-wait NoOps injected just
#    before the instruction on the same engine (same-engine program order makes
#    this equivalent).
# ---------------------------------------------------------------------------
import bass_rust
from bass_rust import ScopedClock


def _patched_drain_and_barrier(self, tick_clock, wait_clock):
    nop_inst = self.nc.sync.nop(nofuse=True, hint="drain_waits")
    wait_clock.add_sem_waits(
        nop_inst.ins, ScopedClock({None: tick_clock.global_clock})
    )
    si = nop_inst.ins.sync_info
    waits = list(si.on_wait) if si is not None else []
    if si is not None:
        si.on_wait = waits[:1]
    for w in waits[1:]:
        extra = self.nc.sync.nop(nofuse=True, hint="drain_waits")
        extra.ins.sync_info = bass_rust.SyncInfo(on_wait=[w], on_update=[])
    self.nc.sync.drain()
    self.nc.all_engine_barrier()
    assert self.sems is not None
    popped = self.nc._tile_sem_poison_stack.pop()
    assert popped is self._sem_poison
    self.nc.clear_and_free_semaphores(list(self.sems.allocated().values()))
    self.nc.all_engine_barrier()


tile.TileContext._drain_and_barrier = _patched_drain_and_barrier

_orig_to_json_bytes = bass.Bass.to_json_bytes
_WSPLIT_COUNTER = [0]


def _split_multi_waits(mod: dict) -> dict:
    for fn in mod.get("functions", []):
        for blk in fn.get("blocks", []):
            insts = blk.get("instructions")
            if not insts:
                continue
            new_insts = []
            changed = False
            for inst in insts:
                si = inst.get("sync_info") or {}
                waits = si.get("on_wait") or []
                if len(waits) > 1:
                    changed = True
                    for w in waits:
                        _WSPLIT_COUNTER[0] += 1
                        new_insts.append({
                            "name": f"I-wsplit-{_WSPLIT_COUNTER[0]}",
                            "opcode": "NoOp",
                            "engine": inst["engine"],
                            "ins": [],
                            "outs": [],
                            "debug": inst.get("debug"),
                            "sync_info": {"on_wait": [w], "on_update": []},
                        })
                    si = dict(si)
                    si["on_wait"] = []
                    inst = dict(inst)
                    inst["sync_info"] = si
                new_insts.append(inst)
            if changed:
                blk["instructions"] = new_insts
    return mod


def _patched_to_json_bytes(self) -> bytes:
    mod = orjson.loads(_orig_to_json_bytes(self))
    return orjson.dumps(_split_multi_waits(mod))


bass.Bass.to_json_bytes = _patched_to_json_bytes

# ---------------------------------------------------------------------------
# Problem constants (hardcoded per the task contract).
# ---------------------------------------------------------------------------
B, T, C = 2, 2048, 2048
N_HEAD, N_KV, D = 16, 4, 128
WINDOW, SINK, THETA = 1024, 4, 10000.0
SCALE = 1.0 / float(np.sqrt(D))
N_CORES = 8
HPG = N_HEAD // N_KV          # query heads per kv group (4)
NT = T // 128                 # 16 query/key tiles
BF = mybir.dt.bfloat16
F32 = mybir.dt.float32

LAST_RESULT = None            # test harness reads exec_time_ns off this


def _half_kjs(H):
    """Key tiles feeding query half H (8 query tiles). The first entry covers
    the FULL half (kj=0 for H=0 via the window; kj=8 for H=1 via the window)
    so every PSUM accumulation starts there; for H=1 the kj=0 sink/edge tile
    comes second so its exp + mask latency hides behind kj=8's big matmuls."""
    starter = 8 * H
    out = [(starter, 8 * H, 8 * H + 7)]
    for kj in range(NT):
        if kj == starter:
            continue
        if kj == 0:
            # sink tile: visible to the whole upper half (bsmask prunes rows)
            out.append((0, 8 * H, 8 * H + 7))
            continue
        lo, hi = max(kj, 8 * H), min(kj + 8, 8 * H + 7)
        if lo <= hi:
            out.append((kj, lo, hi))
    return out


_PHASES = 3


def _build_program(n_loop=1):
    nc = bass.Bass("TRN2", target_bir_lowering=False, debug=False,
                   num_devices=N_CORES)

    xT_d = nc.declare_dram_parameter("xT", [C, T], BF, isOutput=False)
    wqT_d = nc.declare_dram_parameter("wqT", [C, HPG * D], BF, isOutput=False)
    wkT_d = nc.declare_dram_parameter("wkT", [C, D], BF, isOutput=False)
    wvT_d = nc.declare_dram_parameter("wvT", [C, D], BF, isOutput=False)
    woT_d = nc.declare_dram_parameter("woT", [HPG * D, C], BF, isOutput=False)
    cc_d = nc.declare_dram_parameter("cc", [D, T], BF, isOutput=False)
    ss_d = nc.declare_dram_parameter("ss", [D, T], BF, isOutput=False)
    r_d = nc.declare_dram_parameter("rmat", [D, D], BF, isOutput=False)
    id_d = nc.declare_dram_parameter("ident", [D, D], BF, isOutput=False)
    # 0/1 mask for the kj=0 blocks of the upper query half: block 0 is the
    # window-edge-or-sink pattern for q-tile 8, blocks 1..7 are sink-rows-only.
    bs_d = nc.declare_dram_parameter("bsmask", [D, 1024], BF, isOutput=False)
    outT_d = nc.declare_dram_parameter("outT", [C, T], F32, isOutput=True)

    KT = C // 128  # 16 contraction tiles

    def _emit_body(tc):
        with tc.tile_pool(name="consts", bufs=1) as consts, \
             tc.tile_pool(name="persist", bufs=1) as persist:

            # ---- stage all DRAM inputs into SBUF ----
            xt = [consts.tile([128, T], BF, tag=f"xt{i}", name=f"xt{i}") for i in range(KT)]
            wq_t = [consts.tile([128, HPG * D], BF, tag=f"wq{i}", name=f"wq{i}")
                    for i in range(KT)]
            wk_t = [consts.tile([128, D], BF, tag=f"wk{i}", name=f"wk{i}") for i in range(KT)]
            wv_t = [consts.tile([128, D], BF, tag=f"wv{i}", name=f"wv{i}") for i in range(KT)]
            wo_t = [consts.tile([128, T], BF, tag=f"wo{i}", name=f"wo{i}")
                    for i in range(HPG)]
            cc_sb = consts.tile([D, T], BF, tag="cc", name="cc")
            ss_sb = consts.tile([D, T], BF, tag="ss", name="ss")
            r_sb = consts.tile([D, D], BF, tag="rmat", name="rmat")
            id_sb = consts.tile([D, D], BF, tag="ident", name="ident")
            ones_col = consts.tile([128, 1], BF, tag="ones_col", name="ones_col")
            ones_row = consts.tile([1, 128], F32, tag="ones_row", name="ones_row")
            bs_sb = consts.tile([D, 1024], BF, tag="bsmask", name="bsmask")

            # DMA in consumption order: the k-projection below runs k-outer,
            # so PE starts as soon as (wk[0], xt[0]) land instead of waiting
            # for the full 11 MB of staging.
            for i in range(KT):
                nc.sync.dma_start(out=wk_t[i],
                                  in_=wkT_d[128 * i:128 * i + 128, :])
                nc.sync.dma_start(out=xt[i], in_=xT_d[128 * i:128 * i + 128, :])
                nc.sync.dma_start(out=wv_t[i],
                                  in_=wvT_d[128 * i:128 * i + 128, :])
            for i in range(KT):
                nc.sync.dma_start(out=wq_t[i],
                                  in_=wqT_d[128 * i:128 * i + 128, :])
            nc.sync.dma_start(out=cc_sb, in_=cc_d[:, :])
            nc.sync.dma_start(out=ss_sb, in_=ss_d[:, :])
            nc.sync.dma_start(out=r_sb, in_=r_d[:, :])
            nc.sync.dma_start(out=id_sb, in_=id_d[:, :])
            nc.sync.dma_start(out=bs_sb, in_=bs_d[:, :])
            for m in range(HPG):
                nc.sync.dma_start(out=wo_t[m],
                                  in_=woT_d[128 * m:128 * m + 128, :])
            nc.vector.memset(ones_col, 1.0)
            nc.vector.memset(ones_row, 1.0)

            qT = [persist.tile([128, T], BF, tag=f"qT{h}", name=f"qT{h}") for h in range(HPG)]
            kT = persist.tile([128, T], BF, tag="kT", name="kT")
            vT_raw = persist.tile([128, T], BF, tag="vT_raw", name="vT_raw")
            v_nat = persist.tile([128, T], BF, tag="v_nat", name="v_nat")
            yT = [persist.tile([128, T], BF, tag=f"yT{h}", name=f"yT{h}") for h in range(HPG)]

            # ================= projections + RoPE / v-transpose ============
            NCHUNK = T // 512  # 4

            # k and v projections run contraction-outer, interleaved per
            # x-tile, so the PE consumes (wk, xt, wv) tiles at DMA arrival
            # pace during the ~31us staging window instead of idling. Their
            # 8 accumulators need all 8 PSUM banks, so this pool closes
            # before the rope/transpose pools open.
            kv_raws = []
            with tc.tile_pool(name="kv_ps", bufs=1, space="PSUM") as kvp:
                ps_kv = [kvp.tile([128, 512], F32, tag=f"kv{u}{t}",
                                  name=f"kv{u}{t}")
                         for u in ("k", "v") for t in range(NCHUNK)]
                for ck in range(KT):
                    for i, wt in ((0, wk_t[ck]), (1, wv_t[ck])):
                        for t4 in range(NCHUNK):
                            nc.tensor.matmul(
                                ps_kv[NCHUNK * i + t4], wt,
                                xt[ck][:, 512 * t4:512 * t4 + 512],
                                start=(ck == 0), stop=(ck == KT - 1))
                for i, unit in ((0, "k"), (1, "v")):
                    for t4 in range(NCHUNK):
                        raw = persist.tile([128, 512], BF,
                                           tag=f"raw{i}{t4}",
                                           name=f"raw{i}{t4}")
                        nc.scalar.copy(raw, ps_kv[NCHUNK * i + t4])
                        kv_raws.append((unit, 512 * t4, raw))

            with tc.tile_pool(name="proj_ps", bufs=3, space="PSUM") as pps, \
                 tc.tile_pool(name="rope_ps", bufs=2, space="PSUM") as rps, \
                 tc.tile_pool(name="vt_ps", bufs=2, space="PSUM") as vps, \
                 tc.tile_pool(name="rope_sb", bufs=6) as rsb:

                steps = []
                for unit in [("q", h) for h in range(HPG)]:
                    for ch in range(NCHUNK):
                        steps.append((unit, ch))

                pending = []  # deferred post-processing closures (PE/DVE work)

                def make_post(unit, c0, raw):
                    def post():
                        if unit[0] == "v":
                            vslice = vT_raw[:, c0:c0 + 512]
                            nc.vector.tensor_copy(vslice, raw)
                            for j in range(4):
                                tp = vps.tile([128, 128], BF, tag="vt", name="vt")
                                nc.tensor.transpose(
                                    tp, vT_raw[:, c0 + 128 * j:c0 + 128 * j + 128],
                                    id_sb)
                                nc.vector.tensor_copy(
                                    v_nat[:, c0 + 128 * j:c0 + 128 * j + 128],
                                    tp)
                        else:
                            dst = kT if unit[0] == "k" else qT[unit[1]]
                            rot = rps.tile([128, 512], F32, tag="rot", name="rot")
                            nc.tensor.matmul(rot, r_sb, raw,
                                             start=True, stop=True)
                            t1 = rsb.tile([128, 512], BF, tag="t1", name="t1")
                            nc.vector.tensor_mul(t1, raw, cc_sb[:, c0:c0 + 512])
                            t2 = rsb.tile([128, 512], BF, tag="t2", name="t2")
                            nc.vector.tensor_mul(t2, rot, ss_sb[:, c0:c0 + 512])
                            nc.vector.tensor_add(dst[:, c0:c0 + 512], t1, t2)
                    return post

                for unit, c0, raw in kv_raws:
                    pending.append(make_post((unit,), c0, raw))

                def emit_step(unit, ch):
                    c0 = 512 * ch
                    ps = pps.tile([128, 512], F32, tag="proj", name="proj")
                    for ck in range(KT):
                        h = unit[1]
                        lhsT = wq_t[ck][:, 128 * h:128 * h + 128]
                        nc.tensor.matmul(ps, lhsT, xt[ck][:, c0:c0 + 512],
                                         start=(ck == 0), stop=(ck == KT - 1))
                    raw = rsb.tile([128, 512], BF, tag="raw", name="raw")
                    nc.scalar.copy(raw, ps)  # ACT: psum -> sbuf bf16
                    pending.append(make_post(unit, c0, raw))

                for unit, ch in steps:
                    emit_step(unit, ch)
                    while len(pending) > 2:
                        pending.pop(0)()
                while pending:
                    pending.pop(0)()

            # ========================= attention ===========================
            if _PHASES < 2:
                return
            with tc.tile_pool(name="st_ps", bufs=2, space="PSUM") as sps, \
                 tc.tile_pool(name="yt_ps", bufs=1, space="PSUM") as yps, \
                 tc.tile_pool(name="cs_ps", bufs=1, space="PSUM") as cps, \
                 tc.tile_pool(name="pt_sb", bufs=4) as ptp, \
                 tc.tile_pool(name="ytu_sb", bufs=2) as ytup, \
                 tc.tile_pool(name="norm_sb", bufs=2) as nrm:

                # Deferred normalization tails: each half's recip/scale chain
                # is emitted only after the next half's score matmuls, so the
                # PE never sits behind the ACT Ln/Exp latency, and yt/cs PSUM
                # are released by cheap copies instead of the full chain.
                tails = []

                for h in range(HPG):
                    for H in range(2):
                        q0 = 1024 * H
                        kjs = _half_kjs(H)
                        first_kj = kjs[0][0]
                        last_kj = kjs[-1][0]
                        yt = yps.tile([128, 1024], F32, tag="yt", name="yt")
                        cs = cps.tile([1, 1024], F32, tag="cs", name="cs")

                        pend = []  # deferred colsum+AV for the previous kj

                        for kj, lo, hi in kjs:
                            c0, c1 = lo * 128, (hi + 1) * 128
                            ncols = c1 - c0
                            st = sps.tile([128, 1024], F32, tag="st", name="st")
                            for off in range(0, ncols, 512):
                                w = min(512, ncols - off)
                                nc.tensor.matmul(
                                    st[:, off:off + w],
                                    kT[:, 128 * kj:128 * kj + 128],
                                    qT[h][:, c0 + off:c0 + off + w],
                                    start=True, stop=True)
                            pt = ptp.tile([128, 1024], BF, tag="pt", name="pt")
                            nc.scalar.activation(
                                pt[:, :ncols], st[:, :ncols],
                                mybir.ActivationFunctionType.Exp,
                                bias=0.0, scale=SCALE)
                            # --- masks: zero disallowed entries of exp ---
                            if lo <= kj <= hi:
                                s = (kj - lo) * 128  # causal diag: keep c >= p
                                nc.gpsimd.affine_select(
                                    out=pt[:, s:s + 128], in_=pt[:, s:s + 128],
                                    compare_op=mybir.AluOpType.is_ge,
                                    fill=0.0, base=0,
                                    pattern=[[1, 128]], channel_multiplier=-1)
                            if kj >= 1 and hi == kj + 8:
                                s = (hi - lo) * 128  # window edge: keep p >= c
                                nc.gpsimd.affine_select(
                                    out=pt[:, s:s + 128], in_=pt[:, s:s + 128],
                                    compare_op=mybir.AluOpType.is_ge,
                                    fill=0.0, base=0,
                                    pattern=[[-1, 128]], channel_multiplier=1)
                            if kj == 0 and H == 1:
                                # q-tile 8: keep (p >= c) | (p < 4);
                                # q-tiles 9..15: sink rows only. One 0/1 mask.
                                nc.vector.tensor_mul(pt[:, 0:1024],
                                                     pt[:, 0:1024], bs_sb)

                            def make_post(kj, lo, hi, pt):
                                c0, c1 = lo * 128, (hi + 1) * 128
                                ncols = c1 - c0
                                l0 = c0 - q0

                                def post():
                                    for off in range(0, ncols, 512):
                                        w = min(512, ncols - off)
                                        nc.tensor.matmul(
                                            cs[:, l0 + off:l0 + off + w],
                                            ones_col, pt[:, off:off + w],
                                            start=(kj == first_kj),
                                            stop=(kj == last_kj),
                                            skip_group_check=True)
                                    for off in range(0, ncols, 512):
                                        w = min(512, ncols - off)
                                        nc.tensor.matmul(
                                            yt[:, l0 + off:l0 + off + w],
                                            v_nat[:, 128 * kj:128 * kj + 128],
                                            pt[:, off:off + w],
                                            start=(kj == first_kj),
                                            stop=(kj == last_kj),
                                            skip_group_check=True)
                                return post
                            pend.append(make_post(kj, lo, hi, pt))
                            if len(pend) > 2:
                                pend.pop(0)()
                            if kj == kjs[1][0] and tails:
                                # one kj later than the starter: gives the DVE
                                # reciprocal time to finish before the PE hits
                                # the broadcast outer-product
                                tails.pop(0)()
                        while pend:
                            pend.pop(0)()

                        # Free the PSUM accumulators right away: unnormalized
                        # yT to SBUF (bf16), colsum via the Ln read.
                        ytu = ytup.tile([128, 1024], BF, tag="ytu", name="ytu")
                        nc.vector.tensor_copy(ytu, yt)
                        # 1/s on the DVE (InstReciprocal is supported by this
                        # walrus; ACT Reciprocal is banned and the
                        # reciprocal_approx_* customs don't encode). Reading
                        # cs here also releases the PSUM bank immediately.
                        recip = nrm.tile([1, 1024], F32, tag="recip",
                                         name="recip")
                        nc.vector.reciprocal(recip, cs)

                        def make_tail(h, q0, recip, ytu):
                            def tail():
                                rb_ps = sps.tile([128, 1024], F32, tag="st",
                                                 name="st")
                                for off in (0, 512):
                                    nc.tensor.matmul(rb_ps[:, off:off + 512],
                                                     ones_row,
                                                     recip[:, off:off + 512],
                                                     start=True, stop=True)
                                nc.vector.tensor_mul(yT[h][:, q0:q0 + 1024],
                                                     ytu, rb_ps)
                            return tail
                        tails.append(make_tail(h, q0, recip, ytu))
                while tails:
                    tails.pop(0)()

            # ===================== output projection =======================
            if _PHASES < 3:
                return
            with tc.tile_pool(name="wo_ps", bufs=4, space="PSUM") as wps, \
                 tc.tile_pool(name="out_sb", bufs=4) as osb:
                flip = 0
                for o in range(NT):
                    for n in range(NCHUNK):
                        ps = wps.tile([128, 512], F32, tag="wo", name="wo")
                        for m in range(HPG):
                            nc.tensor.matmul(
                                ps, wo_t[m][:, 128 * o:128 * o + 128],
                                yT[m][:, 512 * n:512 * n + 512],
                                start=(m == 0), stop=(m == HPG - 1))
                        ob = osb.tile([128, 512], F32, tag="ob", name="ob")
                        if flip % 2 == 0:
                            nc.scalar.copy(ob, ps)
                        else:
                            nc.vector.tensor_copy(ob, ps)
                        flip += 1
                        nc.sync.dma_start(
                            out=outT_d[128 * o:128 * o + 128,
                                       512 * n:512 * n + 512],
                            in_=ob)
    with tile.TileContext(nc) as tc:
        if n_loop > 1:
            with tc.For_i(0, n_loop, 1):
                _emit_body(tc)
        else:
            _emit_body(tc)
    return nc


_PROGRAM = None


def _get_program():
    global _PROGRAM
    if _PROGRAM is None:
        _PROGRAM = _build_program()
    return _PROGRAM


def _host_inputs(x, wq, wk, wv, wo):
    bf = ml_dtypes.bfloat16
    inv_freq = 1.0 / (THETA ** (np.arange(0, D, 2, dtype=np.float32) / D))
    ang = np.outer(np.arange(T, dtype=np.float32), inv_freq)  # [T, 64]
    cosT, sinT = np.cos(ang).T, np.sin(ang).T                 # [64, T]
    cc = np.ascontiguousarray(np.concatenate([cosT, cosT], 0).astype(bf))
    ss = np.ascontiguousarray(np.concatenate([-sinT, sinT], 0).astype(bf))
    rmat = np.zeros((D, D), np.float32)
    rmat[np.arange(64), np.arange(64) + 64] = 1.0
    rmat[np.arange(64) + 64, np.arange(64)] = 1.0
    rmat = rmat.astype(bf)
    ident = np.eye(D, dtype=np.float32).astype(bf)
    p = np.arange(128)[:, None]
    c = np.arange(128)[None, :]
    bsmask = np.zeros((128, 1024), np.float32)
    bsmask[:, 0:128] = ((p >= c) | (p < SINK)).astype(np.float32)
    bsmask[0:SINK, 128:1024] = 1.0
    bsmask = np.ascontiguousarray(bsmask.astype(bf))

    xT_by_batch = [np.ascontiguousarray(x[b].T.astype(bf)) for b in range(B)]
    w_by_group = [
        {
            "wqT": np.ascontiguousarray(
                wq[512 * g:512 * g + 512, :].T.astype(bf)),
            "wkT": np.ascontiguousarray(
                wk[128 * g:128 * g + 128, :].T.astype(bf)),
            "wvT": np.ascontiguousarray(
                wv[128 * g:128 * g + 128, :].T.astype(bf)),
            "woT": np.ascontiguousarray(
                wo[:, 512 * g:512 * g + 512].T.astype(bf)),
        }
        for g in range(HPG)
    ]
    in_maps = []
    for core in range(N_CORES):
        b, g = divmod(core, HPG)
        in_maps.append({
            "xT": xT_by_batch[b],
            **w_by_group[g],
            "cc": cc, "ss": ss, "rmat": rmat, "ident": ident,
            "bsmask": bsmask,
        })
    return in_maps


def kernel(x, wq, wk, wv, wo):
    global LAST_RESULT
    x = np.asarray(x, np.float32)
    wq = np.asarray(wq, np.float32)
    wk = np.asarray(wk, np.float32)
    wv = np.asarray(wv, np.float32)
    wo = np.asarray(wo, np.float32)

    nc = _get_program()
    in_maps = _host_inputs(x, wq, wk, wv, wo)
    # NTFF tracing is not available under this container's axon build
    # (antenv.axon_hooks absent) and would crash run_bass_kernel_spmd.
    os.environ["BASS_NEVER_TRACE"] = "1"
    res = run_bass_kernel_spmd(nc, in_maps, list(range(N_CORES)), trace=False)
    LAST_RESULT = res

    out = np.zeros((B, T, C), np.float32)
    for core in range(N_CORES):
        b = core // HPG
        out[b] += np.asarray(res.results[core]["outT"], np.float32).T
    return out



# revision 28
# speedup vs baseline: 1.2770x; 1.2770x over previous
"""GQA causal self-attention (sliding window 1024 + 4-token sink) on 8 trn2
NeuronCores.

Sharding: data parallel on batch (2) x tensor parallel on kv-head groups (4).
Core c handles batch c//4 and kv head c%4 (query heads 4g..4g+3): wq/wk/wv are
split column-wise (rows of the [out,in] weights), wo row-wise; each core
produces a [C,T] partial of the output projection (bf16) and the host sums the
4 partials per batch in fp32.

Per-core kernel:
  - q/k/v projections run as fp8e4 DoubleRow matmuls (2 contraction k-tiles
    per instruction at 0.5 cycles/column) with hi/lo error compensation:
    x = x_hi + x_lo, w = w_hi + w_lo (all e4m3, w pre-scaled by 32 to dodge
    subnormals), computing x_hi*w_hi (tile-paired) + x_hi*w_lo + x_lo*w_hi
    (slot-paired in one DoubleRow each). The dropped x_lo*w_lo term is
    ~0.1% — overall precision is at bf16 level at ~4x the throughput.
  - RoPE applied in [d,t] layout via a half-swap permutation matmul +
    elementwise combine; scores S^T[tj,ti] per 128-wide key tile against the
    sliding window + sink, masked by zeroing exp(S^T) in SBUF (gpsimd
    affine_select / a 0-1 mask multiply), softmax without max-subtraction
    (|scale*S| <= ~6 for this distribution).
  - softmax denominators via near-free stationary-pt matmuls (out free size 1)
    into a [128q, 8] PSUM tile in natural layout; 1/s on DVE; transposed by
    the PE and broadcast across partitions with tiny "expander" matmuls; yT
    column-scaled on DVE.
  - wo row-block matmul emits outT = partial^T in bf16.
"""

import os
import sys

import numpy as np
import ml_dtypes

sys.path.insert(0, "/opt/trn_rl_repo")

import orjson

import concourse.bass as bass
import concourse.tile as tile
from concourse import mybir
from concourse.bass_utils import run_bass_kernel_spmd

# ---------------------------------------------------------------------------
# Workarounds for the walrus build in this container: it rejects more than one
# sync-wait per instruction (setupSyncWait on the *_NO_STRUCT encodings).
# 1) TileContext's final drain carries one wait per live proc -> put each wait
#    on its own NoOp ahead of a clean drain.
# 2) Any scheduled instruction can end up with >1 waits -> post-process the
#    serialized BIR and hoist extra waits onto single-wait NoOps injected just
#    before the instruction on the same engine (same-engine program order makes
#    this equivalent).
# ---------------------------------------------------------------------------
import bass_rust
from bass_rust import ScopedClock


def _patched_drain_and_barrier(self, tick_clock, wait_clock):
    nop_inst = self.nc.sync.nop(nofuse=True, hint="drain_waits")
    wait_clock.add_sem_waits(
        nop_inst.ins, ScopedClock({None: tick_clock.global_clock})
    )
    si = nop_inst.ins.sync_info
    waits = list(si.on_wait) if si is not None else []
    if si is not None:
        si.on_wait = waits[:1]
    for w in waits[1:]:
        extra = self.nc.sync.nop(nofuse=True, hint="drain_waits")
        extra.ins.sync_info = bass_rust.SyncInfo(on_wait=[w], on_update=[])
    self.nc.sync.drain()
    self.nc.all_engine_barrier()
    assert self.sems is not None
    popped = self.nc._tile_sem_poison_stack.pop()
    assert popped is self._sem_poison
    self.nc.clear_and_free_semaphores(list(self.sems.allocated().values()))
    self.nc.all_engine_barrier()


tile.TileContext._drain_and_barrier = _patched_drain_and_barrier

_orig_to_json_bytes = bass.Bass.to_json_bytes
_WSPLIT_COUNTER = [0]


def _split_multi_waits(mod: dict) -> dict:
    for fn in mod.get("functions", []):
        for blk in fn.get("blocks", []):
            insts = blk.get("instructions")
            if not insts:
                continue
            new_insts = []
            changed = False
            for inst in insts:
                si = inst.get("sync_info") or {}
                waits = si.get("on_wait") or []
                if len(waits) > 1:
                    changed = True
                    for w in waits:
                        _WSPLIT_COUNTER[0] += 1
                        new_insts.append({
                            "name": f"I-wsplit-{_WSPLIT_COUNTER[0]}",
                            "opcode": "NoOp",
                            "engine": inst["engine"],
                            "ins": [],
                            "outs": [],
                            "debug": inst.get("debug"),
                            "sync_info": {"on_wait": [w], "on_update": []},
                        })
                    si = dict(si)
                    si["on_wait"] = []
                    inst = dict(inst)
                    inst["sync_info"] = si
                new_insts.append(inst)
            if changed:
                blk["instructions"] = new_insts
    return mod


def _patched_to_json_bytes(self) -> bytes:
    mod = orjson.loads(_orig_to_json_bytes(self))
    return orjson.dumps(_split_multi_waits(mod))


bass.Bass.to_json_bytes = _patched_to_json_bytes

# ---------------------------------------------------------------------------
# Problem constants (hardcoded per the task contract).
# ---------------------------------------------------------------------------
B, T, C = 2, 2048, 2048
N_HEAD, N_KV, D = 16, 4, 128
WINDOW, SINK, THETA = 1024, 4, 10000.0
SCALE = 1.0 / float(np.sqrt(D))
N_CORES = 8
HPG = N_HEAD // N_KV          # query heads per kv group (4)
NT = T // 128                 # 16 query/key tiles
KT = C // 128                 # 16 contraction tiles
NPAIR = KT // 2               # DoubleRow contraction-tile pairs
NCHUNK = T // 512             # 4
WS = 32.0                     # fp8 weight pre-scale (host) / 1/WS on eviction
BF = mybir.dt.bfloat16
F32 = mybir.dt.float32
FP8 = mybir.dt.float8e4
DR = mybir.MatmulPerfMode.DoubleRow

LAST_RESULT = None            # test harness reads exec_time_ns off this


def _half_kjs(H):
    """Key tiles feeding query half H (8 query tiles). The first entry covers
    the FULL half (kj=0 for H=0 via the window; kj=8 for H=1 via the window)
    so every PSUM accumulation starts there; for H=1 the kj=0 sink/edge tile
    comes second so its exp + mask latency hides behind kj=8's big matmuls."""
    starter = 8 * H
    out = [(starter, 8 * H, 8 * H + 7)]
    for kj in range(NT):
        if kj == starter:
            continue
        if kj == 0:
            # sink tile: visible to the whole upper half (bsmask prunes rows)
            out.append((0, 8 * H, 8 * H + 7))
            continue
        lo, hi = max(kj, 8 * H), min(kj + 8, 8 * H + 7)
        if lo <= hi:
            out.append((kj, lo, hi))
    return out


_PHASES = 3


def _build_program(n_loop=1):
    nc = bass.Bass("TRN2", target_bir_lowering=False, debug=False,
                   num_devices=N_CORES)

    xhi_d = nc.declare_dram_parameter("xhiT", [C, T], FP8, isOutput=False)
    xlo_d = nc.declare_dram_parameter("xloT", [C, T], FP8, isOutput=False)
    # weights pre-interleaved on host: [128 part, t(16), s(2: hi,lo), m]
    wq_d = nc.declare_dram_parameter("wqall", [128, KT * 2 * HPG * D], FP8,
                                     isOutput=False)
    wk_d = nc.declare_dram_parameter("wkall", [128, KT * 2 * D], FP8,
                                     isOutput=False)
    wv_d = nc.declare_dram_parameter("wvall", [128, KT * 2 * D], FP8,
                                     isOutput=False)
    wo_d = nc.declare_dram_parameter("woall", [128, HPG * 2 * C], FP8,
                                     isOutput=False)
    cc_d = nc.declare_dram_parameter("cc", [D, T], BF, isOutput=False)
    ss_d = nc.declare_dram_parameter("ss", [D, T], BF, isOutput=False)
    r_d = nc.declare_dram_parameter("rmat", [D, D], BF, isOutput=False)
    id_d = nc.declare_dram_parameter("ident", [D, D], BF, isOutput=False)
    idf_d = nc.declare_dram_parameter("identf32", [D, D], F32, isOutput=False)
    # expander: [8, 1024] with block j = E_j (row j all-ones in cols 128j..)
    ex_d = nc.declare_dram_parameter("expander", [8, 1024], BF, isOutput=False)
    # 0/1 mask for the kj=0 blocks of the upper query half: block 0 is the
    # window-edge-or-sink pattern for q-tile 8, blocks 1..7 are sink-rows-only.
    bs_d = nc.declare_dram_parameter("bsmask", [D, 1024], BF, isOutput=False)
    outT_d = nc.declare_dram_parameter("outT", [C, T], BF, isOutput=True)

    def _emit_body(tc):
        with tc.tile_pool(name="consts", bufs=1) as consts, \
             tc.tile_pool(name="persist", bufs=1) as persist:

            # ---- stage all DRAM inputs into SBUF ----
            xall = consts.tile([128, KT * 2 * T], FP8, tag="xall", name="xall")
            x4 = xall.rearrange("p (t s n) -> p t s n", t=KT, s=2)
            wqall = consts.tile([128, KT * 2 * HPG * D], FP8, tag="wqall",
                                name="wqall")
            wq4 = wqall.rearrange("p (t s m) -> p t s m", t=KT, s=2)
            wkall = consts.tile([128, KT * 2 * D], FP8, tag="wkall",
                                name="wkall")
            wk4 = wkall.rearrange("p (t s m) -> p t s m", t=KT, s=2)
            wvall = consts.tile([128, KT * 2 * D], FP8, tag="wvall",
                                name="wvall")
            wv4 = wvall.rearrange("p (t s m) -> p t s m", t=KT, s=2)
            woall = consts.tile([128, HPG * 2 * C], FP8, tag="woall",
                                name="woall")
            wo4 = woall.rearrange("p (t s m) -> p t s m", t=HPG, s=2)
            # wo's moving operand: y in fp8 hi/lo, [m-tile][s=(lo,hi)][col],
            # written on-chip from yT after each half's normalization
            y8all = persist.tile([128, HPG * 2 * T], FP8, tag="y8all",
                                 name="y8all")
            y84 = y8all.rearrange("p (t s n) -> p t s n", t=HPG, s=2)
            cc_sb = consts.tile([D, T], BF, tag="cc", name="cc")
            ss_sb = consts.tile([D, T], BF, tag="ss", name="ss")
            r_sb = consts.tile([D, D], BF, tag="rmat", name="rmat")
            id_sb = consts.tile([D, D], BF, tag="ident", name="ident")
            idf_sb = consts.tile([D, D], F32, tag="identf32", name="identf32")
            ex_sb = consts.tile([8, 1024], BF, tag="expander", name="expander")
            ones_col = consts.tile([128, 1], BF, tag="ones_col",
                                   name="ones_col")
            zerosL = consts.tile([128, 128], BF, tag="zerosL", name="zerosL")
            zeros8 = consts.tile([128, 8], BF, tag="zeros8", name="zeros8")
            bs_sb = consts.tile([D, 1024], BF, tag="bsmask", name="bsmask")

            # DMA in consumption order: the kv projection below runs
            # pair-outer, so the PE starts as soon as the first weight chunk
            # and x pair land instead of waiting for the full staging. x-hi
            # and weights ride the SP queue; x-lo rides the (otherwise idle)
            # Pool queue and wq the ACT queue so the three streams overlap
            # across DMA engines.
            KPW = 2 * 2 * D          # wk/wv columns per tile pair
            for p in range(NPAIR):
                nc.sync.dma_start(out=wkall[:, KPW * p:KPW * p + KPW],
                                  in_=wk_d[:, KPW * p:KPW * p + KPW])
                # both tiles of a pair in one strided DMA
                nc.sync.dma_start(
                    out=x4[:, 2 * p:2 * p + 2, 1, :],
                    in_=xhi_d[256 * p:256 * p + 256, :].rearrange(
                        "(t p) c -> p t c", t=2))
                nc.sync.dma_start(out=wvall[:, KPW * p:KPW * p + KPW],
                                  in_=wv_d[:, KPW * p:KPW * p + KPW])
                nc.sync.dma_start(
                    out=x4[:, 2 * p:2 * p + 2, 0, :],
                    in_=xlo_d[256 * p:256 * p + 256, :].rearrange(
                        "(t p) c -> p t c", t=2))
            PAIRW = 2 * 2 * HPG * D  # wqall columns per tile pair
            for p in range(NPAIR):
                nc.sync.dma_start(out=wqall[:, PAIRW * p:PAIRW * p + PAIRW],
                                  in_=wq_d[:, PAIRW * p:PAIRW * p + PAIRW])
            nc.sync.dma_start(out=cc_sb, in_=cc_d[:, :])
            nc.sync.dma_start(out=ss_sb, in_=ss_d[:, :])
            nc.sync.dma_start(out=r_sb, in_=r_d[:, :])
            nc.sync.dma_start(out=id_sb, in_=id_d[:, :])
            nc.sync.dma_start(out=idf_sb, in_=idf_d[:, :])
            nc.sync.dma_start(out=ex_sb, in_=ex_d[:, :])
            nc.sync.dma_start(out=bs_sb, in_=bs_d[:, :])
            nc.sync.dma_start(out=woall, in_=wo_d[:, :])
            nc.vector.memset(ones_col, 1.0)
            nc.vector.memset(zerosL, 0.0)
            nc.vector.memset(zeros8, 0.0)

            qT = [persist.tile([128, T], BF, tag=f"qT{h}", name=f"qT{h}")
                  for h in range(HPG)]
            kT = persist.tile([128, T], BF, tag="kT", name="kT")
            vT_raw = persist.tile([128, T], BF, tag="vT_raw", name="vT_raw")
            v_nat = persist.tile([128, T], BF, tag="v_nat", name="v_nat")
            yT = [persist.tile([128, T], BF, tag=f"yT{h}", name=f"yT{h}")
                  for h in range(HPG)]

            # ================= projections + RoPE / v-transpose ============
            # fp8 DoubleRow 3-term: per contraction-tile pair p,
            #   hi-hi:      lhsT (w_hi[2p], w_hi[2p+1]),  rhs (x_hi[2p], x_hi[2p+1])
            #   corr t:     lhsT (w_hi[t], w_lo[t]),      rhs (x_lo[t], x_hi[t])
            # w layout s=(hi,lo), x layout s=(lo,hi) make both patterns clean
            # strided APs into one tile.
            def emit_fp8_proj(dst_ps, w4, msl, cs, p, start, stop):
                nc.tensor.matmul(dst_ps, w4[:, 2 * p:2 * p + 2, 0, msl],
                                 x4[:, 2 * p:2 * p + 2, 1, cs],
                                 start=start, stop=False, perf_mode=DR)
                for t in (2 * p, 2 * p + 1):
                    nc.tensor.matmul(dst_ps, w4[:, t, :, msl],
                                     x4[:, t, :, cs],
                                     start=False,
                                     stop=(stop and t == 2 * p + 1),
                                     perf_mode=DR)

            # k and v projections run pair-outer, interleaved per pair, so the
            # PE consumes tiles at DMA arrival pace during the staging window.
            # Their 8 accumulators need all 8 PSUM banks, so this pool closes
            # before the rope/transpose pools open.
            kv_raws = []
            with tc.tile_pool(name="kv_ps", bufs=1, space="PSUM") as kvp:
                ps_kv = [kvp.tile([128, 512], F32, tag=f"kv{u}{t}",
                                  name=f"kv{u}{t}")
                         for u in ("k", "v") for t in range(NCHUNK)]

                def evict_kv(i, unit, t4, on_dve):
                    raw = persist.tile([128, 512], BF, tag=f"raw{i}{t4}",
                                       name=f"raw{i}{t4}")
                    if on_dve:
                        nc.vector.tensor_scalar_mul(
                            raw, ps_kv[NCHUNK * i + t4], 1.0 / WS)
                    else:
                        nc.scalar.activation(
                            raw, ps_kv[NCHUNK * i + t4],
                            mybir.ActivationFunctionType.Copy,
                            bias=0.0, scale=1.0 / WS)
                    kv_raws.append((unit, 512 * t4, raw))

                for p in range(NPAIR):
                    last = p == NPAIR - 1
                    for i, w4_ in ((0, wk4), (1, wv4)):
                        for t4 in range(NCHUNK):
                            cs = slice(512 * t4, 512 * t4 + 512)
                            emit_fp8_proj(ps_kv[NCHUNK * i + t4], w4_,
                                          slice(0, D), cs, p,
                                          start=(p == 0), stop=last)
                        if last:
                            # evict this unit right away (k's banks free
                            # while v's last matmuls still run), alternating
                            # ACT/DVE so neither engine queues 4 deep
                            for t4 in range(NCHUNK):
                                evict_kv(i, "k" if i == 0 else "v", t4,
                                         on_dve=(t4 % 2 == 1))

            with tc.tile_pool(name="proj_ps", bufs=3, space="PSUM") as pps, \
                 tc.tile_pool(name="rope_ps", bufs=2, space="PSUM") as rps, \
                 tc.tile_pool(name="vt_ps", bufs=2, space="PSUM") as vps, \
                 tc.tile_pool(name="rope_sb", bufs=6) as rsb:

                # head 3's q projection is NOT emitted here: it rides inside
                # the H=0 attention halves as PE filler (it is only needed by
                # the 4th half), shortening the DMA-bound projection window.
                steps = []
                for unit in [("q", h) for h in range(HPG - 1)]:
                    for ch in range(NCHUNK):
                        steps.append((unit, ch))

                pending = []  # deferred post-processing closures (PE/DVE work)

                def make_post(unit, c0, raw):
                    def post():
                        if unit[0] == "v":
                            vslice = vT_raw[:, c0:c0 + 512]
                            nc.vector.tensor_copy(vslice, raw)
                            for j in range(4):
                                tp = vps.tile([128, 128], BF, tag="vt",
                                              name="vt")
                                nc.tensor.transpose(
                                    tp,
                                    vT_raw[:, c0 + 128 * j:c0 + 128 * j + 128],
                                    id_sb)
                                nc.vector.tensor_copy(
                                    v_nat[:, c0 + 128 * j:c0 + 128 * j + 128],
                                    tp)
                        else:
                            dst = kT if unit[0] == "k" else qT[unit[1]]
                            rot = rps.tile([128, 512], F32, tag="rot",
                                           name="rot")
                            nc.tensor.matmul(rot, r_sb, raw,
                                             start=True, stop=True)
                            t1 = rsb.tile([128, 512], BF, tag="t1", name="t1")
                            nc.vector.tensor_mul(t1, raw, cc_sb[:, c0:c0 + 512])
                            t2 = rsb.tile([128, 512], BF, tag="t2", name="t2")
                            nc.vector.tensor_mul(t2, rot, ss_sb[:, c0:c0 + 512])
                            nc.vector.tensor_add(dst[:, c0:c0 + 512], t1, t2)
                    return post

                for unit, c0, raw in kv_raws:
                    pending.append(make_post((unit,), c0, raw))

                def emit_step(unit, ch):
                    c0 = 512 * ch
                    cs = slice(c0, c0 + 512)
                    h = unit[1]
                    msl = slice(128 * h, 128 * h + 128)
                    ps = pps.tile([128, 512], F32, tag="proj", name="proj")
                    for p in range(NPAIR):
                        emit_fp8_proj(ps, wq4, msl, cs, p,
                                      start=(p == 0), stop=(p == NPAIR - 1))
                    raw = rsb.tile([128, 512], BF, tag="raw", name="raw")
                    nc.scalar.activation(raw, ps,
                                         mybir.ActivationFunctionType.Copy,
                                         bias=0.0, scale=1.0 / WS)
                    pending.append(make_post(unit, c0, raw))

                for unit, ch in steps:
                    emit_step(unit, ch)
                    while len(pending) > 2:
                        pending.pop(0)()
                while pending:
                    pending.pop(0)()

            # ========================= attention ===========================
            if _PHASES < 2:
                return
            wo_chunks = [(o, n) for n in range(2) for o in range(NT)]
            with tc.tile_pool(name="st_ps", bufs=2, space="PSUM") as sps, \
                 tc.tile_pool(name="yt_ps", bufs=1, space="PSUM") as yps, \
                 tc.tile_pool(name="den_ps", bufs=1, space="PSUM") as dps, \
                 tc.tile_pool(name="woi_ps", bufs=1, space="PSUM") as wip, \
                 tc.tile_pool(name="pt_sb", bufs=4) as ptp, \
                 tc.tile_pool(name="ytu_sb", bufs=2) as ytup, \
                 tc.tile_pool(name="woi_sb", bufs=2) as wis, \
                 tc.tile_pool(name="h3_sb", bufs=2) as h3p, \
                 tc.tile_pool(name="norm_sb", bufs=4) as nrm:

                # wo chunks for T columns 0:1024 become ready once the H=0
                # halves and their tails are done; they are emitted as PE
                # filler work inside the H=1 halves so the PE has something
                # to chew on while ACT exp paces the softmax pipeline.
                # fp8 DoubleRow over (m-tile, d) with on-chip y hi/lo; the
                # 1/(32*32) for the w and y pre-scales folds into eviction.
                def emit_wo_matmuls(ps, o, n):
                    osl = slice(128 * o, 128 * o + 128)
                    cs = slice(512 * n, 512 * n + 512)
                    for p in range(HPG // 2):
                        nc.tensor.matmul(ps, wo4[:, 2 * p:2 * p + 2, 0, osl],
                                         y84[:, 2 * p:2 * p + 2, 1, cs],
                                         start=(p == 0), stop=False,
                                         perf_mode=DR)
                        for t in (2 * p, 2 * p + 1):
                            nc.tensor.matmul(
                                ps, wo4[:, t, :, osl], y84[:, t, :, cs],
                                start=False,
                                stop=(p == HPG // 2 - 1 and t == 2 * p + 1),
                                perf_mode=DR)

                def emit_wo_chunk():
                    o, n = wo_chunks.pop(0)
                    ps = wip.tile([128, 512], F32, tag="woi", name="woi")
                    emit_wo_matmuls(ps, o, n)
                    ob = wis.tile([128, 512], BF, tag="ob", name="ob")
                    # DVE eviction (ACT is the attention floor), 1/1024 scale
                    nc.vector.tensor_scalar_mul(ob, ps, 1.0 / (WS * WS))
                    nc.sync.dma_start(
                        out=outT_d[128 * o:128 * o + 128,
                                   512 * n:512 * n + 512],
                        in_=ob)

                # head-3 q-projection chunks, interleaved into the H=0 halves
                # (emit_chunk at one kj, its rope post a kj later so the PE
                # never waits on the DVE eviction)
                def make_h3_chunk(ch):
                    st_ = {}
                    c0 = 512 * ch
                    cs = slice(c0, c0 + 512)

                    def emit_chunk():
                        ps = wip.tile([128, 512], F32, tag="woi", name="woi")
                        for p in range(NPAIR):
                            emit_fp8_proj(ps, wq4, slice(384, 512), cs, p,
                                          start=(p == 0),
                                          stop=(p == NPAIR - 1))
                        raw = h3p.tile([128, 512], BF, tag="h3raw",
                                       name="h3raw")
                        nc.vector.tensor_scalar_mul(raw, ps, 1.0 / WS)
                        st_["raw"] = raw

                    def emit_rope():
                        raw = st_["raw"]
                        rot = wip.tile([128, 512], F32, tag="woi", name="woi")
                        nc.tensor.matmul(rot, r_sb, raw, start=True, stop=True)
                        t1 = h3p.tile([128, 512], BF, tag="h3t1", name="h3t1")
                        nc.vector.tensor_mul(t1, raw, cc_sb[:, cs])
                        t2 = h3p.tile([128, 512], BF, tag="h3t2", name="h3t2")
                        nc.vector.tensor_mul(t2, rot, ss_sb[:, cs])
                        nc.vector.tensor_add(qT[3][:, cs], t1, t2)
                    return emit_chunk, emit_rope

                h3_fillers = {0: [], 1: [], 2: []}
                for ch in range(NCHUNK):
                    cfn, rfn = make_h3_chunk(ch)
                    tgt = 0 if ch < 2 else ch - 1
                    h3_fillers[tgt] += [cfn, rfn]

                # Deferred normalization tails. Each tail has two stages:
                # stage a (PE transpose of 1/den into an st-rotation carrier
                # + a DVE eviction of the transposed row) fires one kj into
                # the next half so the DVE reciprocal is done; stage b
                # (expander broadcast matmuls into the same carrier + DVE
                # column scale) fires one kj later.
                tails = []  # [dict(a=, b=, a_done=bool)]

                half_idx = 0
                for H in range(2):
                    for h in range(HPG):
                        q0 = 1024 * H
                        kjs = _half_kjs(H)
                        first_kj = kjs[0][0]
                        last_kj = kjs[-1][0]
                        # per-local-subtile first/last kj for den accumulation
                        sub_first, sub_last = {}, {}
                        for kj, lo, hi in kjs:
                            for j in range(lo - 8 * H, hi - 8 * H + 1):
                                sub_first.setdefault(j, kj)
                                sub_last[j] = kj
                        yt = yps.tile([128, 1024], F32, tag="yt", name="yt")
                        # A start=True matmul zeroes the whole 2KB PSUM bank,
                        # so the bank is cleared ONCE by this dummy matmul and
                        # every real den matmul accumulates with start=False.
                        den = dps.tile([128, 8], F32, tag="den", name="den")
                        nc.tensor.matmul(den, zerosL, zeros8,
                                         start=True, stop=False,
                                         skip_group_check=True)
                        half_idx += 1

                        pend = []  # deferred den-colsum+AV for previous kj

                        for ki, (kj, lo, hi) in enumerate(kjs):
                            c0, c1 = lo * 128, (hi + 1) * 128
                            ncols = c1 - c0
                            st = sps.tile([128, 1024], F32, tag="st",
                                          name="st")
                            for off in range(0, ncols, 512):
                                w = min(512, ncols - off)
                                nc.tensor.matmul(
                                    st[:, off:off + w],
                                    kT[:, 128 * kj:128 * kj + 128],
                                    qT[h][:, c0 + off:c0 + off + w],
                                    start=True, stop=True)
                            pt = ptp.tile([128, 1024], BF, tag="pt", name="pt")
                            nc.scalar.activation(
                                pt[:, :ncols], st[:, :ncols],
                                mybir.ActivationFunctionType.Exp,
                                bias=0.0, scale=SCALE)
                            # --- masks: zero disallowed entries of exp ---
                            if lo <= kj <= hi:
                                s = (kj - lo) * 128  # causal diag: keep c >= p
                                nc.gpsimd.affine_select(
                                    out=pt[:, s:s + 128], in_=pt[:, s:s + 128],
                                    compare_op=mybir.AluOpType.is_ge,
                                    fill=0.0, base=0,
                                    pattern=[[1, 128]], channel_multiplier=-1)
                            if kj >= 1 and hi == kj + 8:
                                s = (hi - lo) * 128  # window edge: keep p >= c
                                nc.gpsimd.affine_select(
                                    out=pt[:, s:s + 128], in_=pt[:, s:s + 128],
                                    compare_op=mybir.AluOpType.is_ge,
                                    fill=0.0, base=0,
                                    pattern=[[-1, 128]], channel_multiplier=1)
                            if kj == 0 and H == 1:
                                # q-tile 8: keep (p >= c) | (p < 4);
                                # q-tiles 9..15: sink rows only. One 0/1 mask.
                                nc.vector.tensor_mul(pt[:, 0:1024],
                                                     pt[:, 0:1024], bs_sb)

                            def make_post(kj, lo, hi, pt):
                                c0, c1 = lo * 128, (hi + 1) * 128
                                ncols = c1 - c0
                                l0 = c0 - q0

                                def post():
                                    for off in range(0, ncols, 512):
                                        w = min(512, ncols - off)
                                        nc.tensor.matmul(
                                            yt[:, l0 + off:l0 + off + w],
                                            v_nat[:, 128 * kj:128 * kj + 128],
                                            pt[:, off:off + w],
                                            start=(kj == first_kj),
                                            stop=(kj == last_kj),
                                            skip_group_check=True)
                                    # denominators: stationary-pt matmuls with
                                    # a [128,1] ones moving operand — out free
                                    # size 1, so the PE cost is ~nil. start is
                                    # NEVER set (the bank was pre-zeroed).
                                    for j128 in range(0, ncols, 128):
                                        jl = (c0 + j128) // 128 - 8 * H
                                        nc.tensor.matmul(
                                            den[:, jl:jl + 1],
                                            pt[:, j128:j128 + 128],
                                            ones_col,
                                            start=False,
                                            stop=(kj == sub_last[jl]),
                                            skip_group_check=True)
                                return post
                            pend.append(make_post(kj, lo, hi, pt))
                            if len(pend) > 2:
                                pend.pop(0)()
                            if ki == 1 and tails and not tails[0]["a_done"]:
                                tails[0]["a"]()
                                tails[0]["a_done"] = True
                            if ki == 2 and tails:
                                tl = tails.pop(0)
                                if not tl["a_done"]:
                                    tl["a"]()
                                tl["b"]()
                            # PE filler between softmax pipeline steps
                            if ki >= 2:
                                if H == 0:
                                    fl = h3_fillers.get(half_idx - 1)
                                    if fl:
                                        fl.pop(0)()
                                elif wo_chunks:
                                    emit_wo_chunk()
                        while pend:
                            pend.pop(0)()

                        # Free the yt PSUM accumulator right away:
                        # unnormalized yT to SBUF (bf16), 1/den via DVE
                        # reciprocal.
                        ytu = ytup.tile([128, 1024], BF, tag="ytu", name="ytu")
                        nc.vector.tensor_copy(ytu, yt)
                        recip = nrm.tile([128, 8], F32, tag="recip",
                                         name="recip")
                        nc.vector.reciprocal(recip, den)

                        def make_tail(h, q0, recip, ytu):
                            st_ = {}

                            def stage_a():
                                # transpose 1/den into the corner of an
                                # st-rotation carrier; the expander broadcast
                                # in stage b reuses the same carrier
                                ca = sps.tile([128, 1024], F32, tag="st",
                                              name="st")
                                nc.tensor.transpose(ca[0:8, 0:128], recip,
                                                    idf_sb)
                                rT = nrm.tile([8, 128], BF, tag="rT",
                                              name="rT")
                                nc.vector.tensor_copy(rT, ca[0:8, 0:128])
                                st_["ca"] = ca
                                st_["rT"] = rT

                            def stage_b():
                                ca, rT = st_["ca"], st_["rT"]
                                # start only at bank boundaries (j=0, j=4):
                                # start zeroes the whole 2KB bank, which would
                                # wipe already-written sibling blocks
                                for j in range(8):
                                    nc.tensor.matmul(
                                        ca[:, 128 * j:128 * j + 128],
                                        ex_sb[:, 128 * j:128 * j + 128],
                                        rT, start=(j % 4 == 0),
                                        stop=(j % 4 == 3),
                                        skip_group_check=True)
                                nc.vector.tensor_mul(yT[h][:, q0:q0 + 1024],
                                                     ytu, ca)
                                # wo's fp8 hi/lo copies of this yT half (yT
                                # carries a 32x pre-scale from the expander)
                                ys = yT[h][:, q0:q0 + 1024]
                                nc.scalar.activation(
                                    y84[:, h, 1, q0:q0 + 1024], ys,
                                    mybir.ActivationFunctionType.Copy,
                                    bias=0.0, scale=1.0)
                                nc.vector.tensor_sub(
                                    y84[:, h, 0, q0:q0 + 1024], ys,
                                    y84[:, h, 1, q0:q0 + 1024])
                            return {"a": stage_a, "b": stage_b,
                                    "a_done": False}
                        tails.append(make_tail(h, q0, recip, ytu))
                while tails:
                    tl = tails.pop(0)
                    if not tl["a_done"]:
                        tl["a"]()
                    tl["b"]()
                    if wo_chunks:
                        emit_wo_chunk()

            # ===================== output projection =======================
            if _PHASES < 3:
                return
            with tc.tile_pool(name="wo_ps", bufs=6, space="PSUM") as wps, \
                 tc.tile_pool(name="out_sb", bufs=4) as osb:
                flip = 0
                while wo_chunks:  # leftovers not placed during attention
                    o, n = wo_chunks.pop(0)
                    ps = wps.tile([128, 512], F32, tag="wo", name="wo")
                    emit_wo_matmuls(ps, o, n)
                    obs = osb.tile([128, 1024], BF, tag="ob", name="ob")
                    nc.vector.tensor_scalar_mul(obs[:, 0:512], ps,
                                                1.0 / (WS * WS))
                    nc.sync.dma_start(
                        out=outT_d[128 * o:128 * o + 128,
                                   512 * n:512 * n + 512],
                        in_=obs[:, 0:512])
                # T columns 1024:2048, two chunks per o-row batched into one
                # eviction tile and a single DMA (fewer descriptors => no
                # descriptor-generation backlog at the drain)
                for o in range(NT):
                    ob = osb.tile([128, 1024], BF, tag="ob", name="ob")
                    for idx, n in enumerate((2, 3)):
                        ps = wps.tile([128, 512], F32, tag="wo", name="wo")
                        emit_wo_matmuls(ps, o, n)
                        dst = ob[:, 512 * idx:512 * idx + 512]
                        if flip % 2 == 0:
                            nc.scalar.activation(
                                dst, ps, mybir.ActivationFunctionType.Copy,
                                bias=0.0, scale=1.0 / (WS * WS))
                        else:
                            nc.vector.tensor_scalar_mul(dst, ps,
                                                        1.0 / (WS * WS))
                        flip += 1
                    nc.sync.dma_start(
                        out=outT_d[128 * o:128 * o + 128, 1024:2048],
                        in_=ob)
    with tile.TileContext(nc) as tc:
        if n_loop > 1:
            with tc.For_i(0, n_loop, 1):
                _emit_body(tc)
        else:
            _emit_body(tc)
    return nc


_PROGRAM = None


def _get_program():
    global _PROGRAM
    if _PROGRAM is None:
        _PROGRAM = _build_program()
    return _PROGRAM


def _interleave_hi_lo(w, n_out):
    """[rows, n_out] f32 -> [128, (rows/128)*2*n_out] fp8 in
    [p, (t, s=(hi,lo), m)] layout."""
    f8 = ml_dtypes.float8_e4m3
    nt = w.shape[0] // 128
    w = w * WS
    w_hi = w.astype(f8)
    w_lo = (w - w_hi.astype(np.float32)).astype(f8)
    hi = w_hi.reshape(nt, 128, n_out)   # [t, p, m]
    lo = w_lo.reshape(nt, 128, n_out)
    st = np.stack([hi, lo], axis=1)     # [t, s, p, m]
    return np.ascontiguousarray(
        st.transpose(2, 0, 1, 3).reshape(128, nt * 2 * n_out))


def _host_inputs(x, wq, wk, wv, wo):
    bf = ml_dtypes.bfloat16
    f8 = ml_dtypes.float8_e4m3
    inv_freq = 1.0 / (THETA ** (np.arange(0, D, 2, dtype=np.float32) / D))
    ang = np.outer(np.arange(T, dtype=np.float32), inv_freq)  # [T, 64]
    cosT, sinT = np.cos(ang).T, np.sin(ang).T                 # [64, T]
    cc = np.ascontiguousarray(np.concatenate([cosT, cosT], 0).astype(bf))
    ss = np.ascontiguousarray(np.concatenate([-sinT, sinT], 0).astype(bf))
    rmat = np.zeros((D, D), np.float32)
    rmat[np.arange(64), np.arange(64) + 64] = 1.0
    rmat[np.arange(64) + 64, np.arange(64)] = 1.0
    rmat = rmat.astype(bf)
    ident = np.eye(D, dtype=np.float32).astype(bf)
    identf32 = np.eye(D, dtype=np.float32)
    # the expander carries the 32x pre-scale applied to y before its fp8
    # split for the wo matmul (undone by the 1/1024 output eviction scale)
    expander = np.zeros((8, 1024), np.float32)
    for j in range(8):
        expander[j, 128 * j:128 * j + 128] = WS
    expander = np.ascontiguousarray(expander.astype(bf))
    p = np.arange(128)[:, None]
    c = np.arange(128)[None, :]
    bsmask = np.zeros((128, 1024), np.float32)
    bsmask[:, 0:128] = ((p >= c) | (p < SINK)).astype(np.float32)
    bsmask[0:SINK, 128:1024] = 1.0
    bsmask = np.ascontiguousarray(bsmask.astype(bf))

    xhi_by_batch, xlo_by_batch = [], []
    for b in range(B):
        xT = np.ascontiguousarray(x[b].T)
        xhi = xT.astype(f8)
        xlo = (xT - xhi.astype(np.float32)).astype(f8)
        xhi_by_batch.append(xhi)
        xlo_by_batch.append(xlo)

    w_by_group = [
        {
            "wqall": _interleave_hi_lo(
                np.ascontiguousarray(wq[512 * g:512 * g + 512, :].T), 512),
            "wkall": _interleave_hi_lo(
                np.ascontiguousarray(wk[128 * g:128 * g + 128, :].T), 128),
            "wvall": _interleave_hi_lo(
                np.ascontiguousarray(wv[128 * g:128 * g + 128, :].T), 128),
            "woall": _interleave_hi_lo(
                np.ascontiguousarray(wo[:, 512 * g:512 * g + 512].T), C),
        }
        for g in range(HPG)
    ]
    in_maps = []
    for core in range(N_CORES):
        b, g = divmod(core, HPG)
        in_maps.append({
            "xhiT": xhi_by_batch[b],
            "xloT": xlo_by_batch[b],
            **w_by_group[g],
            "cc": cc, "ss": ss, "rmat": rmat, "ident": ident,
            "identf32": identf32, "expander": expander,
            "bsmask": bsmask,
        })
    return in_maps


def kernel(x, wq, wk, wv, wo):
    global LAST_RESULT
    x = np.asarray(x, np.float32)
    wq = np.asarray(wq, np.float32)
    wk = np.asarray(wk, np.float32)
    wv = np.asarray(wv, np.float32)
    wo = np.asarray(wo, np.float32)

    nc = _get_program()
    in_maps = _host_inputs(x, wq, wk, wv, wo)
    # NTFF tracing is not available under this container's axon build
    # (antenv.axon_hooks absent) and would crash run_bass_kernel_spmd.
    os.environ["BASS_NEVER_TRACE"] = "1"
    res = run_bass_kernel_spmd(nc, in_maps, list(range(N_CORES)), trace=False)
    LAST_RESULT = res

    out = np.zeros((B, T, C), np.float32)
    for core in range(N_CORES):
        b = core // HPG
        out[b] += np.asarray(res.results[core]["outT"]).astype(np.float32).T
    return out
